# revision 17
# baseline (speedup 1.0000x reference)
"""Trainium2 Bass kernel for nn_Classifier (segment_reduce).

Computation (reference):
    local  = relu(x @ W1.T)            # [T, 50] @ [50, 400] -> [T, 400]
    feat   = mean over windows of J=24 # [T//24, 400]
    logits = feat @ W2.T               # [T//24, 400] @ [400, 10]

Strategy: pure data parallel over 8 NeuronCores (x sharded along T).
Per core (T_c = 98304 rows = 4096 windows):
  - Host packs the x shard TRANSPOSED + bf16 into xp [128, 49152]:
    rows 0-49 hold x_shard[:49152].T, rows 64-113 hold x_shard[49152:].T.
    This puts the contraction dim (n=50) on partitions so matmul1 needs no
    on-device transpose, and the two shard halves row-tile the PE array
    (tile_position (0,0) / (64,0)) for 2x concurrent matmuls.
  - matmul1: lhsT = xp[:, 128-col tile] (stationary), rhs = W1.T [50, 400]
    -> psum [128t, 400k] fp32; two tiles packed per [128, 1024] psum pair.
  - relu evacuation psum->sbuf bf16 split 10:8:6 across ACT/DVE/POOL.
  - pooling runs on the PE with FLIPPED operands: rl tile chunk
    [128tau, 100k] is the stationary, the 0/1 window matrix [128tau, 16w]
    is the moving operand, so each matmul costs 16 output columns instead
    of 400; output featT [100k, 16w] accumulates over the 3 tiles of a
    384-row group, and is already transposed for matmul2.
  - per 4-group block (64 windows): featT [100, 4, 64] evacuates to sbuf
    and matmul2 contracts the four 100-row k-chunks of W2.T/24 straight
    into logits psum [128w, 10]; one [128, 10] DMA out per shard-half.
"""

import sys

sys.path.insert(0, "/opt/trn_rl_repo")

import numpy as np
import ml_dtypes

import bass_rust
import concourse.bass as bass
import concourse.mybir as mybir
import concourse.tile as tile
from concourse.bass_utils import run_bass_kernel_spmd
from concourse.tile import TileContext
from concourse.vector_clock import ScopedClock

# ---------------------------------------------------------------------------
# Wait-count legalization (monkeypatch).
#
# This walrus build accepts at most 1 sync-wait per instruction (2 for
# EventSemaphore), but Tile's scheduler and tail drain can attach more,
# failing codegen with "Too many sync wait commands". Spread excess waits
# onto same-engine NOPs inserted immediately before the instruction.
# ---------------------------------------------------------------------------

_orig_add = TileContext._add_instruction


def _wait_cap(inst):
    return 2 if type(inst).__name__ == "InstEventSemaphore" else 1


def _patched_add_instruction(self, inst):
    si = inst.sync_info
    cap = _wait_cap(inst)
    if (
        si is not None
        and si.on_wait
        and len(si.on_wait) > cap
        and inst.engine != mybir.EngineType.Unassigned
    ):
        waits = list(si.on_wait)
        for w in waits[:-cap]:
            nop = bass_rust.InstNoOp(
                name=f"I-waitfix-{self.nc.next_id()}",
                opcode="NoOp",
                engine=inst.engine,
                ins=[],
                outs=[],
            )
            nop.sync_info = mybir.SyncInfo(on_wait=[w], on_update=[])
            _orig_add(self, nop)
        inst.sync_info = mybir.SyncInfo(
            on_wait=waits[-cap:], on_update=list(si.on_update or [])
        )
    _orig_add(self, inst)


def _patched_drain_and_barrier(self, tick_clock, wait_clock):
    nc = self.nc
    drain_inst = nc.sync.drain()
    wait_clock.add_sem_waits(
        drain_inst.ins, ScopedClock({None: tick_clock.global_clock})
    )
    mi = drain_inst.ins
    si = mi.sync_info
    waits = list(si.on_wait) if (si and si.on_wait) else []
    if len(waits) > 1:
        mi.sync_info = mybir.SyncInfo(
            on_wait=[waits[-1]], on_update=list(si.on_update or [])
        )
        for w in waits[:-1]:
            nop = nc.sync.nop()
            nop.ins.sync_info = mybir.SyncInfo(on_wait=[w], on_update=[])

    nc.all_engine_barrier()
    assert self.sems is not None
    popped = nc._tile_sem_poison_stack.pop()
    assert popped is self._sem_poison
    nc.clear_and_free_semaphores(list(self.sems.allocated().values()))
    nc.all_engine_barrier()


TileContext._add_instruction = _patched_add_instruction
TileContext._drain_and_barrier = _patched_drain_and_barrier

# ---------------------------------------------------------------------------
# Problem constants (hardcoded per the harness contract)
# ---------------------------------------------------------------------------

J = 24
T, N, K, C = 786432, 50, 400, 10
NCORES = 8
TC = T // NCORES          # 98304 rows per core
H = TC // 2               # 49152 cols per half in xp
B_CORE = TC // J          # 4096 windows per core
NTILE = H // 128          # 384 tiles of 128 rows per half
NG = 16                   # supergroup iterations (8 groups x 3 tiles each)
CHUNK = 24 * 128          # 3072 xp columns per supergroup

BF16 = mybir.dt.bfloat16
F32 = mybir.dt.float32
nbf = ml_dtypes.bfloat16

# evac engine rotation: GPSIMD cannot access PSUM, so evacuation is
# ACT/DVE only; 12:12 with the small tail ops (fts, lsb) going to ACT.
EVAC_PAT = ["A", "D"]


def _build_pmat3():
    """Pooling matrices as matmul MOVING operand: [128, 48], col block
    16*r + w.  P_r[tau, w] = 1 where w = (128*r + tau) // 24 is the window
    of row 128*r+tau within a 384-row (3-tile) group."""
    pm = np.zeros((128, 48), np.float32)
    for r in range(3):
        for tau in range(128):
            w = (128 * r + tau) // 24
            pm[tau, 16 * r + w] = 1.0
    return pm.astype(nbf)


def _build_nc(repeat: int = 1):
    """repeat>1 re-runs the whole computation in one NEFF — used by the
    test harness to measure device time differentially (wall(R=3) -
    wall(R=1))/2 without NTFF profiling."""
    nc = bass.Bass()
    xp_d = nc.declare_dram_parameter("xp", [128, H], BF16, isOutput=False)
    w1t_d = nc.declare_dram_parameter("w1t", [50, 400], BF16, isOutput=False)
    w2tp_d = nc.declare_dram_parameter("w2tp", [100, 40], BF16, isOutput=False)
    pm_d = nc.declare_dram_parameter("pmat3", [128, 48], BF16, isOutput=False)
    out_d = nc.declare_dram_parameter("logits", [B_CORE, 10], F32, isOutput=True)

    act = mybir.ActivationFunctionType

    with TileContext(nc) as tc:
        with (
            tc.tile_pool(name="consts", bufs=1) as cpool,
            tc.tile_pool(name="xchunks", bufs=3) as xpool,
            tc.tile_pool(name="relu", bufs=26) as rpool,
            tc.tile_pool(name="small", bufs=2) as spool,
            tc.tile_pool(name="mm1ps", bufs=3, space="PSUM") as mm1pool,
            tc.tile_pool(name="featps", bufs=1, space="PSUM") as featpool,
            tc.tile_pool(name="tailps", bufs=1, space="PSUM") as tailpool,
        ):
            # W1T staged at partition offsets 0 and 64 — the moving operand
            # must share the stationary's base partition (array row offset).
            w1t = cpool.tile([128, 400], BF16)
            w2tp = cpool.tile([100, 40], BF16)
            pmat3 = cpool.tile([128, 48], BF16)
            nc.sync.dma_start(out=w1t[0:50, :], in_=w1t_d[:])
            nc.sync.dma_start(out=w1t[64:114, :], in_=w1t_d[:])
            nc.sync.dma_start(out=w2tp[:], in_=w2tp_d[:])
            nc.sync.dma_start(out=pmat3[:], in_=pm_d[:])

            evac_ct = 0
            for G in [g for _ in range(repeat) for g in range(NG)]:
                xc = xpool.tile([128, CHUNK], BF16, name="xc")
                nc.sync.dma_start(
                    out=xc[:], in_=xp_d[:, G * CHUNK : (G + 1) * CHUNK]
                )

                featT = None  # both halves: [100, 2*4, 64] psum (1 bank)
                lps = tailpool.tile([128, 32], F32, name="lps")
                for m8 in range(8):  # 3-tile groups within the supergroup
                    blk, mm = divmod(m8, 4)
                    if mm == 0:
                        featT = featpool.tile([100, 8, 64], F32, name="ftp")
                    # ---- matmul1 + relu evacuation (3 tile-pairs) ----
                    rls = []
                    for j_ in range(3):
                        tcol = (m8 * 3 + j_) * 128
                        ps = mm1pool.tile([128, 1024], F32, name="ps")
                        for hh in range(2):
                            rb = 64 * hh
                            nc.tensor.matmul(
                                ps[:, 512 * hh : 512 * hh + 400],
                                xc[rb : rb + 50, tcol : tcol + 128],
                                w1t[rb : rb + 50, :],
                                start=True,
                                stop=True,
                            )
                        rl = rpool.tile([128, 2, 400], BF16, name="rl", bufs=26)
                        src = ps[:, :].rearrange("p (two k) -> p two k", two=2)[
                            :, :, 0:400
                        ]
                        if EVAC_PAT[evac_ct % 2] == "A":
                            nc.scalar.activation(rl[:], src, act.Relu)
                        else:
                            nc.vector.tensor_scalar_max(rl[:], src, 0.0)
                        evac_ct += 1
                        rls.append(rl)

                    # ---- pooling: rl stationary, pmat moving ----
                    # featT[k, 4*hh + kc, 16*mm + w] += rl_r[tau, hh, k]
                    for hh in range(2):
                        for kc in range(4):
                            for r in range(3):
                                nc.tensor.matmul(
                                    featT[
                                        :, 4 * hh + kc, 16 * mm : 16 * mm + 16
                                    ],
                                    rls[r][:, hh, 100 * kc : 100 * kc + 100],
                                    pmat3[:, 16 * r : 16 * r + 16],
                                    start=(r == 0),
                                    stop=(r == 2),
                                )

                    if mm == 3:  # block of 4 groups (64 windows) complete
                        fts = spool.tile([100, 8, 64], BF16, name="fts")
                        nc.scalar.activation(fts[:], featT[:], act.Relu)
                        # matmul2: contract k chunks; windows on psum
                        # partitions 64*blk .. 64*blk+64, cols 16*hh
                        for hh in range(2):
                            for kc in range(4):
                                nc.tensor.matmul(
                                    lps[
                                        64 * blk : 64 * blk + 64,
                                        16 * hh : 16 * hh + 10,
                                    ],
                                    fts[:, 4 * hh + kc, :],
                                    w2tp[:, 10 * kc : 10 * kc + 10],
                                    start=(kc == 0),
                                    stop=(kc == 3),
                                    tile_position=(0, 64 * blk),
                                )

                # ---- logits evac + output (one DMA per half) ----
                lsb = spool.tile([128, 2, 10], F32, name="lsb")
                nc.scalar.copy(
                    lsb[:],
                    lps[:, :].rearrange("p (h c) -> p h c", h=2)[:, :, 0:10],
                )
                for hh in range(2):
                    rowbase = hh * (B_CORE // 2) + G * 128
                    nc.sync.dma_start(
                        out=out_d[rowbase : rowbase + 128, :],
                        in_=lsb[:, hh, :],
                    )
    return nc


_NC = {}


def _get_nc(repeat: int = 1):
    if repeat not in _NC:
        _NC[repeat] = _build_nc(repeat)
    return _NC[repeat]


def prepare_in_maps(x: np.ndarray, W1: np.ndarray, W2: np.ndarray):
    assert x.shape == (T, N) and W1.shape == (K, N) and W2.shape == (C, K)

    w1t = np.ascontiguousarray(W1.T.astype(nbf))          # [50, 400]
    w2tp = np.ascontiguousarray(
        (W2.T.astype(np.float32) / J).reshape(4, 100, 10).transpose(1, 0, 2)
        .reshape(100, 40)
    ).astype(nbf)                                          # [100, 4*10]
    pmat3 = _build_pmat3()

    xb = x.astype(nbf)
    in_maps = []
    for c in range(NCORES):
        shard = xb[c * TC : (c + 1) * TC]                  # [98304, 50]
        xp = np.zeros((128, H), nbf)
        xp[0:50] = shard[0:H].T
        xp[64:114] = shard[H:].T
        in_maps.append(
            {
                "xp": xp,
                "w1t": w1t,
                "w2tp": w2tp,
                "pmat3": pmat3,
            }
        )
    return in_maps


def kernel(x: np.ndarray, W1: np.ndarray, W2: np.ndarray) -> np.ndarray:
    in_maps = prepare_in_maps(x, W1, W2)
    nc = _get_nc()
    res = run_bass_kernel_spmd(nc, in_maps, core_ids=list(range(NCORES)))
    out = np.concatenate(
        [res.results[c]["logits"] for c in range(NCORES)], axis=0
    )
    return out.astype(np.float32)

